# revision 1
# baseline (speedup 1.0000x reference)
"""Trainium2 Bass kernel for PVT-style spatial-reduction attention.

Reference computation (B=4, N=4096, C=512, 8 heads, head_dim=64):
  q = (x @ q_w.T) * hd**-0.5                    -> [B, N, C]
  x_ = depthwise_conv2x2_stride2(x as NCHW 64x64) + sr_b -> [B, M=1024, C]
  x_ = layernorm(x_) * ln_g + ln_b
  k, v = split(x_ @ kv_w.T)                      -> [B, nh, M, hd] each
  out = softmax(q k^T) v                         -> [B, N, C]
  out = out @ proj_w.T + proj_b

Sharding: 8 cores, core d handles batch b=d//2, query-half h=d%2 (2048
queries).  Each core computes its output slice independently (the small KV
path is recomputed per half); the host concatenates.  No collectives.

Host-side folds: ln_g is folded into k_w/v_w columns; ln_b folds exactly
into proj_b (softmax shift-invariance kills the K-side bias, and the
V-side bias times sum(P)/s == v_w@ln_b is constant per channel, which
commutes through the projection).  The conv input arrives pre-shuffled
into 4 stride-2 tap planes so the depthwise conv is 4 contiguous
multiply-accumulates per c-tile (bf16, DVE 2x mode).

Engine plan: DVE runs conv + LN-normalize + KV psum drains + softmax-sum
accumulation (shared with GpSimd) + 1/s (reciprocal_approx_fast, spread
to partitions 0/32 by the ones-matmul sum chains) + epilogue multiplies.
ACT runs only q^T psum drains, LN stats, and the per-mt Exp.  PE order:
Q proj (interleaved with LN sum matmuls), K proj (all pairs), V proj,
then attention ch-outer/pair-inner with the output projection for each
512-query column group interleaved right after its last pair finishes.
Chunk epilogues (sum matmuls, reciprocal, normalize-broadcast matmuls,
OT write) are deferred into the next chunk's mt stream so no PE stall
sits on the critical path.
"""

import os

import numpy as np

KSTAGE = int(os.environ.get("KSTAGE", "99"))

B, N, C = 4, 4096, 512
NH, HD = 8, 64
M = 1024          # (64/2) * (64/2) spatial-reduced tokens
NHALF = 2048      # queries per core
LN_EPS = 1e-5

NQT = NHALF // 128
KT = C // 128     # 4 c-tiles
MT = M // 128     # 8 m-tiles
NCH = NHALF // 512

_cache = {}


def _build_nc():
    import concourse.tile as tile
    from concourse import bacc, mybir

    f32 = mybir.dt.float32
    f32r = mybir.dt.float32r
    bf16 = mybir.dt.bfloat16
    f8 = mybir.dt.float8e4
    AF = mybir.ActivationFunctionType
    OP = mybir.AluOpType
    PM = mybir.MatmulPerfMode

    # Pin Exp/Ln/Square to the one ACT table set that contains all three
    # (natural_log_exp_and_others); otherwise the set chooser alternates
    # between sets and pays a ~1.3us ACT_TABLE_LOAD per switch in the hot
    # loop.  Indices of the sets are preserved (walrus maps by index).
    import concourse.bacc as bacc_mod
    if not hasattr(bacc_mod, "_orig_get_activation_tables"):
        bacc_mod._orig_get_activation_tables = bacc_mod.get_activation_tables

        def _pinned_tables(arch):
            d = bacc_mod._orig_get_activation_tables(arch)
            strip = {AF.Exp, AF.Ln, AF.Square}
            return {
                name: (funcs if name == "natural_log_exp_and_others"
                       else funcs - strip)
                for name, funcs in d.items()
            }

        bacc_mod.get_activation_tables = _pinned_tables

    nc = bacc.Bacc("TRN2", target_bir_lowering=False, debug=False)

    xc_d = nc.dram_tensor("xc", [C, 4 * M], bf16, kind="ExternalInput")
    xq_d = nc.dram_tensor("xqT", [C, NHALF], bf16, kind="ExternalInput")
    qw_d = nc.dram_tensor("q_wT", [C, C], bf16, kind="ExternalInput")
    kw_d = nc.dram_tensor("k_wT", [C, C], bf16, kind="ExternalInput")
    vw_d = nc.dram_tensor("v_wT", [C, 2 * C], bf16, kind="ExternalInput")
    pw_d = nc.dram_tensor("p_wT", [C, C], bf16, kind="ExternalInput")
    chan_d = nc.dram_tensor("chan", [C, 8], f32, kind="ExternalInput")
    pb_d = nc.dram_tensor("p_b", [1, C], f32, kind="ExternalInput")
    out_d = nc.dram_tensor("out", [NHALF, C], bf16,
                           kind="ExternalOutput")

    with tile.TileContext(nc) as tc:
        _cms = {}

        def pool(name, bufs=1, space="SBUF", side=None):
            cm = tc.tile_pool(name=name, bufs=bufs, space=space, side=side)
            p = cm.__enter__()
            _cms[id(p)] = cm
            return p

        def close(*pools):
            for p in pools:
                _cms.pop(id(p)).__exit__(None, None, None)

        consts = pool("consts")
        ones_f = consts.tile([128, 128], f32, tag="ones_f", name="ones_f")
        nc.vector.memset(ones_f[:], 1.0)
        ones128 = consts.tile([128, 128], bf16, tag="ones128", name="ones128")
        nc.vector.tensor_copy(ones128[:], ones_f[:])
        onesPV = consts.tile([128, 1], bf16, tag="onesPV", name="onesPV")
        nc.vector.tensor_copy(onesPV[:], ones_f[:, 0:1])
        # [ones(64)|zeros(64)] / [zeros(64)|ones(64)] mask rows (all 128
        # partitions identical) for the normalize-broadcast matmuls (f32r)
        mask_f = consts.tile([128, 128], f32, tag="mask_f", name="mask_f")
        nc.vector.memset(mask_f[:], 1.0)
        nc.vector.memset(mask_f[:, 64:128], 0.0)
        maskAr = consts.tile([128, 128], f32r, tag="maskAr", name="maskAr")
        nc.vector.tensor_copy(maskAr[:], mask_f[:])
        nc.vector.memset(mask_f[:, 0:64], 0.0)
        nc.vector.memset(mask_f[:, 64:128], 1.0)
        maskBr = consts.tile([128, 128], f32r, tag="maskBr", name="maskBr")
        nc.vector.tensor_copy(maskBr[:], mask_f[:])
        zero_t = consts.tile([128, 1], f32, tag="zero_t", name="zero_t")
        nc.vector.memset(zero_t[:], 0.0)
        eps_t = consts.tile([128, 1], f32, tag="eps_t", name="eps_t")
        nc.vector.memset(eps_t[:], LN_EPS)
        # chan params: cols 0-3 conv taps, 4 sr_b
        chan = [consts.tile([128, 8], f32, tag=f"chan{k}", name=f"chan{k}")
                for k in range(KT)]
        pb_bc = consts.tile([128, C], f32, tag="pb", name="pb")
        warm = consts.tile([128, 1], f32, tag="warm", name="warm")
        nc.scalar.activation(warm[:], zero_t[:], AF.Exp, bias=zero_t[:])

        # =============== DMA issue (spread across engine queues) ==========
        # sync: xq (ch-major so Q proj can start after ~5 issues) + weights
        # pool: conv planes (k-major)   scalar: chan + pb broadcast
        xc_pool = pool("xcp", side="right")
        xc_sb = xc_pool.tile([128, KT * 4 * M], bf16, tag="xc", name="xc")
        xq_pool = pool("xqp", side="right")
        xq_sb = xq_pool.tile([128, KT * NHALF], bf16, tag="xq", name="xq")
        w_pool = pool("wp")
        qw = [w_pool.tile([128, C], bf16, tag=f"qw{k}", name=f"qw{k}")
              for k in range(KT)]
        kw = [w_pool.tile([128, C], bf16, tag=f"kw{k}", name=f"kw{k}")
              for k in range(KT)]
        vw = [w_pool.tile([128, 2 * C], bf16, tag=f"vw{k}", name=f"vw{k}")
              for k in range(KT)]
        pw = [w_pool.tile([128, C], bf16, tag=f"pw{k}", name=f"pw{k}")
              for k in range(KT)]

        for k in range(KT):
            nc.scalar.dma_start(chan[k][:],
                                chan_d.ap()[128 * k:128 * (k + 1), :])
        for k in range(KT):
            for t in range(0, 4, 2):
                nc.scalar.dma_start(
                    xc_sb[:, k * 4 * M + t * M:k * 4 * M + (t + 2) * M],
                    xc_d.ap()[128 * k:128 * (k + 1), t * M:(t + 2) * M])
        nc.scalar.dma_start(pb_bc[:], pb_d.ap().to_broadcast([128, C]))

        for k in range(KT):
            nc.sync.dma_start(
                xq_sb[:, k * NHALF:k * NHALF + 512],
                xq_d.ap()[128 * k:128 * (k + 1), 0:512])
        for k in range(KT):
            nc.sync.dma_start(qw[k][:], qw_d.ap()[128 * k:128 * (k + 1), :])
        for ch in range(1, NCH):
            for k in range(KT):
                nc.sync.dma_start(
                    xq_sb[:, k * NHALF + 512 * ch:k * NHALF + 512 * (ch + 1)],
                    xq_d.ap()[128 * k:128 * (k + 1), 512 * ch:512 * (ch + 1)])
        for k in range(KT):
            nc.sync.dma_start(kw[k][:], kw_d.ap()[128 * k:128 * (k + 1), :])
        for k in range(KT):
            nc.sync.dma_start(vw[k][:], vw_d.ap()[128 * k:128 * (k + 1), :])
        for k in range(KT):
            nc.sync.dma_start(pw[k][:], pw_d.ap()[128 * k:128 * (k + 1), :])


        # persistent SBUF pools (opened before the short-lived conv pools so
        # the pool stack stays LIFO)
        qt_pool = pool("qtp")
        qT = [qt_pool.tile([128, NHALF], bf16, tag=f"qT{k}", name=f"qT{k}")
              for k in range(KT)]
        zp = pool("zp")
        z = [zp.tile([128, M], bf16, tag=f"z{k}", name=f"z{k}")
             for k in range(KT)]
        kvp = pool("kvp")
        kTt = [kvp.tile([128, M], bf16, tag=f"kT{p}", name=f"kT{p}")
               for p in range(KT)]
        # vt[mt]: [m-tile, 2C] packed per pair p: cols 256p..+127 = [v_A|0],
        # cols 256p+128..+255 = [0|v_B]  (zero-padding baked into v_wT on host)
        vt = [kvp.tile([128, 2 * C], bf16, tag=f"vt{m}", name=f"vt{m}")
              for m in range(MT)]

        # =============== phase C: depthwise conv (DVE, bf16 2x) ===========
        ytp = pool("ytp")
        yt = [ytp.tile([128, M], bf16, tag=f"yt{k}", name=f"yt{k}")
              for k in range(KT)]
        ysq = [ytp.tile([128, M], bf16, tag=f"ysq{k}", name=f"ysq{k}")
               for k in range(KT)]
        cv = pool("cv", bufs=2)
        for k in range(KT):
            acc = cv.tile([128, M], bf16, tag="cacc", name="cacc")
            base = k * 4 * M
            nc.vector.tensor_scalar_mul(
                acc[:], xc_sb[:, base:base + M], chan[k][:, 0:1])
            for t in range(1, 4):
                nc.vector.scalar_tensor_tensor(
                    acc[:], xc_sb[:, base + t * M:base + (t + 1) * M],
                    chan[k][:, t:t + 1], acc[:],
                    op0=OP.mult, op1=OP.add)
            nc.vector.tensor_scalar_add(yt[k][:], acc[:], chan[k][:, 4:5])
            nc.vector.tensor_tensor(ysq[k][:], yt[k][:], yt[k][:], op=OP.mult)

        # =============== phase Q: q^T (PE) + LN sums interleaved ==========
        q_psum = pool("q_ps", bufs=4, space="PSUM")
        st_psum = pool("st_ps", space="PSUM")
        SY = st_psum.tile([128, M], f32, tag="SY", name="SY")
        SY2 = st_psum.tile([128, M], f32, tag="SY2", name="SY2")
        for co in range(KT):
            for ch in range(NCH):
                ps = q_psum.tile([128, 512], f32, tag="q", name="q")
                for k in range(KT):
                    nc.tensor.matmul(
                        ps[:],
                        qw[k][:, 128 * co:128 * (co + 1)],
                        xq_sb[:, k * NHALF + 512 * ch:
                                 k * NHALF + 512 * (ch + 1)],
                        start=(k == 0), stop=(k == KT - 1),
                    )
                nc.scalar.copy(qT[co][:, 512 * ch:512 * (ch + 1)], ps[:])
            # LN sum matmuls for k-tile `co`, interleaved so the PE reaches
            # them right about when the conv (DVE) has produced yt/ysq.
            k = co
            for ch2 in range(M // 512):
                s_ = slice(512 * ch2, 512 * (ch2 + 1))
                nc.tensor.matmul(SY[:, s_], ones128[:], yt[k][:, s_],
                                 start=(k == 0), stop=(k == KT - 1))
                nc.tensor.matmul(SY2[:, s_], ones128[:], ysq[k][:, s_],
                                 start=(k == 0), stop=(k == KT - 1))

        # =============== LN stats + normalize (z = (y-mu)*inv_std) ========
        mu2 = cv.tile([128, M], f32, tag="stat", name="mu2")
        nc.scalar.activation(mu2[:], SY[:], AF.Square, bias=zero_t[:],
                             scale=1.0 / C)
        var = cv.tile([128, M], f32, tag="stat", name="var")
        nc.vector.scalar_tensor_tensor(
            var[:], SY2[:], 1.0 / C, mu2[:], op0=OP.mult, op1=OP.subtract)
        lgv = cv.tile([128, M], f32, tag="stat", name="lgv")
        nc.scalar.activation(lgv[:], var[:], AF.Ln, bias=eps_t[:])
        invb = cv.tile([128, M], bf16, tag="invb", name="invb")
        nc.scalar.activation(invb[:], lgv[:], AF.Exp, bias=zero_t[:],
                             scale=-0.5)
        mu_sb = cv.tile([128, M], bf16, tag="mu", name="mu_sb")
        nc.vector.tensor_scalar_mul(mu_sb[:], SY[:], 1.0 / C)
        for k in range(KT):
            t1 = cv.tile([128, M], bf16, tag="t1", name="t1")
            nc.vector.tensor_tensor(t1[:], yt[k][:], mu_sb[:], op=OP.subtract)
            nc.vector.tensor_tensor(z[k][:], t1[:], invb[:], op=OP.mult)
        close(st_psum, q_psum, cv, ytp, xq_pool, xc_pool)

        # =============== attention-era PSUM pools ==========================
        # S: QK scores (2x[128,1024] = 4 banks; also hosts K-proj chains and
        #    the drain projection chains)   O: PV accum + V-proj chains
        # sb: softmax-sum rows (partitions 0/32) and normalize-broadcast
        S_psum = pool("S_ps", bufs=2, space="PSUM")
        O_psum = pool("O_ps", bufs=2, space="PSUM")
        sb_psum = pool("sb_ps", bufs=2, space="PSUM")

        # V-proj chains (borrow the S pool; woven into chunk 0's mt stream
        # so the PE keeps streaming while early QKs wait on exp)
        def emit_vchain(mt):
            for vh in range(2):
                vs = slice(512 * vh, 512 * (vh + 1))
                vps = S_psum.tile([128, 1024], f32, tag="S", name="vps")
                for k in range(KT):
                    nc.tensor.matmul(
                        vps[:, 0:512], z[k][:, 128 * mt:128 * (mt + 1)],
                        vw[k][:, vs],
                        start=(k == 0), stop=(k == KT - 1))
                nc.vector.tensor_copy(vt[mt][:, vs], vps[:, 0:512])

        def emit_kchain(p):
            for ch2 in range(M // 512):
                s_ = slice(512 * ch2, 512 * (ch2 + 1))
                kps = S_psum.tile([128, 1024], f32, tag="S", name="kps")
                for k in range(KT):
                    nc.tensor.matmul(
                        kps[:, 0:512], kw[k][:, 128 * p:128 * (p + 1)],
                        z[k][:, s_],
                        start=(k == 0), stop=(k == KT - 1))
                nc.vector.tensor_copy(kTt[p][:, s_], kps[:, 0:512])

        if KSTAGE >= 1:
            emit_kchain(0)
            emit_vchain(0)
            emit_vchain(1)

        # =============== phase A: attention, ch outer / pair inner ========
        OT_pool = pool("OTp", side="right")
        OT = [OT_pool.tile([128, NHALF], bf16, tag=f"OT{p}", name=f"OT{p}")
              for p in range(KT)]
        ss_pool = pool("ssp", side="right", bufs=2)
        sinv_pool = pool("sinvp", bufs=2)
        ppool = pool("ptile", bufs=4)
        opool = pool("outp", bufs=3)

        pending_pv = None

        def emit_pv(pv):
            pp, mt, pt, o_ps = pv
            nc.tensor.matmul(
                o_ps[:], vt[mt][:, 256 * pp:256 * pp + 128],
                pt[:, 0:512],
                start=(mt == 0), stop=False)
            nc.tensor.matmul(
                o_ps[:], vt[mt][:, 256 * pp + 128:256 * (pp + 1)],
                pt[:, 512:1024],
                start=False, stop=(mt == MT - 1))

        def emit_smm(st):
            # softmax-sum chains for a finished chunk: head A -> partition 0,
            # head B -> partition 32 of one PSUM bank (so the DVE reciprocal
            # runs lane-parallel and feeds the broadcast matmuls directly).
            accE, accO = st["accE"], st["accO"]
            nc.vector.tensor_tensor(accE[:], accE[:], accO[:], op=OP.add)
            s_t = sb_psum.tile([128, 512], f32, tag="sb", name="s_t")
            nc.tensor.matmul(s_t[0:1, :], onesPV[:], accE[:, 0:512],
                             start=True, stop=True)
            nc.tensor.matmul(s_t[32:33, :], onesPV[:], accE[:, 512:1024],
                             start=True, stop=True, skip_group_check=True)
            st["s_t"] = s_t

        def emit_recip(st):
            # one contiguous call covering partitions 0..32 (rows 1-31 are
            # never-read garbage; single-row calls at partition offset 32
            # mis-evaluate, and strided partition APs fail codegen)
            sinv = sinv_pool.tile([33, 512], f32, tag="sinv", name="sinv")
            nc.vector.reciprocal_approx_fast(sinv[0:33, :],
                                             st["s_t"][0:33, :])
            sinr = sinv_pool.tile([33, 512], f32r, tag="sinr", name="sinr")
            nc.vector.tensor_copy(sinr[0:1, :], sinv[0:1, :])
            nc.vector.tensor_copy(sinr[32:33, :], sinv[32:33, :])
            st["sinr"] = sinr

        def emit_bc(st):
            sinr = st["sinr"]
            bc = sb_psum.tile([128, 512], f32, tag="sb", name="bc")
            nc.tensor.matmul(bc[:], maskAr[0:1, :], sinr[0:1, :],
                             start=True, stop=False)
            nc.tensor.matmul(bc[:], maskBr[32:33, :], sinr[32:33, :],
                             start=False, stop=True, skip_group_check=True)
            st["bc"] = bc

        def emit_ot(st):
            p, ch = st["p"], st["ch"]
            nsl = slice(512 * ch, 512 * (ch + 1))
            nc.vector.tensor_copy(OT[p][:, nsl], st["o_ps"][:])
            nc.vector.tensor_tensor(OT[p][:, nsl], OT[p][:, nsl],
                                    st["bc"][:], op=OP.mult)

        def emit_pj_nt(nt):
            # output projection + bias + store for one 128-query n-tile
            pj = S_psum.tile([128, 1024], f32, tag="S", name="pj")
            for p2 in range(KT):
                nc.tensor.matmul(
                    pj[:, 0:512], OT[p2][:, 128 * nt:128 * (nt + 1)],
                    pw[p2][:],
                    start=(p2 == 0), stop=(p2 == KT - 1))
            ob = opool.tile([128, 512], bf16, tag="ob", name="ob")
            nc.vector.tensor_tensor(ob[:], pj[:, 0:512], pb_bc[:],
                                    op=OP.add)
            nc.sync.dma_start(out_d.ap()[128 * nt:128 * (nt + 1), :],
                              ob[:])

        actions = {}     # mt -> list of thunks, for the current chunk
        n_ch = 0 if KSTAGE < 2 else min(KSTAGE - 1, NCH)
        for ch in range(n_ch):
            for p in range(KT):
                st = {"p": p, "ch": ch,
                      "o_ps": O_psum.tile([128, 512], f32, tag="o",
                                          name="o_ps"),
                      "accE": ss_pool.tile([128, 1024], bf16, tag="accE",
                                           name="accE"),
                      "accO": ss_pool.tile([128, 1024], bf16, tag="accO",
                                           name="accO")}
                nsl = slice(512 * ch, 512 * (ch + 1))
                for mt in range(MT):
                    for fn in actions.pop(mt, ()):
                        fn()
                    msl = slice(128 * mt, 128 * (mt + 1))
                    S_ps = S_psum.tile([128, 1024], f32, tag="S",
                                       name="S_ps")
                    nc.tensor.matmul(
                        S_ps[:, 0:512],
                        kTt[p][0:64, msl], qT[p][0:64, nsl],
                        start=True, stop=True, tile_position=(0, 0))
                    nc.tensor.matmul(
                        S_ps[:, 512:1024],
                        kTt[p][64:128, msl], qT[p][64:128, nsl],
                        start=True, stop=True, tile_position=(64, 0))
                    pt = ppool.tile([128, 1024], bf16, tag="pt", name="pt")
                    nc.scalar.activation(pt[:], S_ps[:], AF.Exp,
                                         bias=zero_t[:])
                    if pending_pv is not None:
                        emit_pv(pending_pv)
                    pending_pv = (p, mt, pt, st["o_ps"])
                    if mt == 0:
                        nc.vector.tensor_copy(st["accE"][:], pt[:])
                    elif mt == 1:
                        nc.vector.tensor_copy(st["accO"][:], pt[:])
                    elif mt % 2 == 0:
                        nc.gpsimd.tensor_tensor(st["accE"][:], st["accE"][:],
                                                pt[:], op=OP.add)
                    else:
                        nc.vector.tensor_tensor(st["accO"][:], st["accO"][:],
                                                pt[:], op=OP.add)
                    if ch == 0 and p == 0 and mt < 6:
                        emit_vchain(mt + 2)
                    if ch == 0 and p < KT - 1 and mt == 5:
                        emit_kchain(p + 1)
                actions = {
                    2: [lambda st=st: emit_smm(st)],
                    3: [lambda st=st: emit_recip(st)],
                    5: [lambda st=st: emit_bc(st)],
                    6: [lambda st=st: emit_ot(st)],
                }
                if p == KT - 1:
                    actions[7] = [lambda nt=nt: emit_pj_nt(nt)
                                  for nt in range(4 * ch, 4 * ch + 4)]
        # drain: last chunk's PV + epilogue + all projection groups
        if pending_pv is not None:
            emit_pv(pending_pv)
        for mt in sorted(actions):
            for fn in actions[mt]:
                fn()
        close(opool, ppool, sinv_pool,
              sb_psum, O_psum, S_psum,
              ss_pool, OT_pool,
              kvp, zp, qt_pool, w_pool, consts)

    nc.compile()
    return nc


def _get_nc():
    if "nc" not in _cache:
        _cache["nc"] = _build_nc()
    return _cache["nc"]


def _make_in_maps(x, q_w, kv_w, proj_w, proj_b, sr_w, sr_b, ln_g, ln_b):
    import ml_dtypes
    bf = ml_dtypes.bfloat16

    x = np.asarray(x, np.float32)
    q_w = np.asarray(q_w, np.float32)
    kv_w = np.asarray(kv_w, np.float32)
    proj_w = np.asarray(proj_w, np.float32)
    g = np.asarray(ln_g, np.float32)
    lb = np.asarray(ln_b, np.float32)
    scale = HD ** -0.5

    q_wT = np.ascontiguousarray((q_w.T * scale).astype(bf))
    # fold ln_g into the kv weight columns (k = k_w @ (z*g + ln_b))
    k_wT = np.ascontiguousarray((kv_w[:C] * g[None, :]).T.astype(bf))
    v_wg = kv_w[C:] * g[None, :]
    v_wT_raw = v_wg.T  # [c_in, c_out=h*64+d]
    # packed per head pair p: [v_A|0] then [0|v_B] (for PV psum packing)
    v_wT = np.zeros((C, 2 * C), np.float32)
    for p in range(KT):
        v_wT[:, 256 * p:256 * p + 64] = v_wT_raw[:, 128 * p:128 * p + 64]
        v_wT[:, 256 * p + 192:256 * p + 256] = \
            v_wT_raw[:, 128 * p + 64:128 * p + 128]
    v_wT = np.ascontiguousarray(v_wT.astype(bf))
    p_wT = np.ascontiguousarray(proj_w.T.astype(bf))
    chan = np.zeros((C, 8), np.float32)
    sr_w = np.asarray(sr_w, np.float32)
    for di in range(2):
        for dj in range(2):
            chan[:, di * 2 + dj] = sr_w[:, 0, di, dj]
    chan[:, 4] = np.asarray(sr_b, np.float32)
    # ln_b folds: K-side bias is killed by softmax shift-invariance; the
    # V-side bias contributes exactly v_w@ln_b per channel (sum(P)/s == 1),
    # which commutes through the projection into an output bias.
    vb = kv_w[C:] @ lb
    p_b = (np.asarray(proj_b, np.float32) + proj_w @ vb).reshape(1, C)

    in_maps = []
    for d in range(8):
        b, half = d // 2, d % 2
        xbT = x[b].T  # [C, N]
        xCHW = xbT.reshape(C, 64, 64)
        planes = np.stack([xCHW[:, di::2, dj::2].reshape(C, M)
                           for di in range(2) for dj in range(2)], axis=1)
        xc = np.ascontiguousarray(planes.reshape(C, 4 * M).astype(bf))
        in_maps.append({
            "xc": xc,
            "xqT": np.ascontiguousarray(
                xbT[:, half * NHALF:(half + 1) * NHALF].astype(bf)),
            "q_wT": q_wT, "k_wT": k_wT, "v_wT": v_wT, "p_wT": p_wT,
            "chan": chan, "p_b": p_b,
        })
    return in_maps


def _assemble(res):
    out = np.empty((B, N, C), np.float32)
    for d in range(8):
        b, half = d // 2, d % 2
        out[b, half * NHALF:(half + 1) * NHALF, :] = res.results[d]["out"]
    return out


def kernel(x, q_w, kv_w, proj_w, proj_b, sr_w, sr_b, ln_g, ln_b, H, W):
    from concourse.bass_utils import run_bass_kernel_spmd

    nc = _get_nc()
    in_maps = _make_in_maps(x, q_w, kv_w, proj_w, proj_b, sr_w, sr_b,
                            ln_g, ln_b)
    res = run_bass_kernel_spmd(nc, in_maps, core_ids=list(range(8)))
    return _assemble(res)



# revision 31
# speedup vs baseline: 1.0151x; 1.0151x over previous
"""Trainium2 Bass kernel for PVT-style spatial-reduction attention.

Reference computation (B=4, N=4096, C=512, 8 heads, head_dim=64):
  q = (x @ q_w.T) * hd**-0.5                    -> [B, N, C]
  x_ = depthwise_conv2x2_stride2(x as NCHW 64x64) + sr_b -> [B, M=1024, C]
  x_ = layernorm(x_) * ln_g + ln_b
  k, v = split(x_ @ kv_w.T)                      -> [B, nh, M, hd] each
  out = softmax(q k^T) v                         -> [B, N, C]
  out = out @ proj_w.T + proj_b

Sharding: 8 cores, core d handles batch b=d//2, query-half h=d%2 (2048
queries).  Each core computes its output slice independently (the small KV
path is recomputed per half); the host concatenates.  No collectives.

Host-side folds: ln_g is folded into k_w/v_w columns; ln_b folds exactly
into proj_b (softmax shift-invariance kills the K-side bias, and the
V-side bias times sum(P)/s == v_w@ln_b is constant per channel, which
commutes through the projection).  The conv input arrives pre-shuffled
into 4 stride-2 tap planes so the depthwise conv is 4 contiguous
multiply-accumulates per c-tile (bf16, DVE 2x mode).

Engine plan: DVE runs conv + LN-normalize + KV psum drains + softmax-sum
accumulation (shared with GpSimd) + 1/s (reciprocal_approx_fast, spread
to partitions 0/32 by the ones-matmul sum chains) + epilogue multiplies.
ACT runs only q^T psum drains, LN stats, and the per-mt Exp.  PE order:
Q proj (interleaved with LN sum matmuls), K proj (all pairs), V proj,
then attention ch-outer/pair-inner with the output projection for each
512-query column group interleaved right after its last pair finishes.
Chunk epilogues (sum matmuls, reciprocal, normalize-broadcast matmuls,
OT write) are deferred into the next chunk's mt stream so no PE stall
sits on the critical path.
"""

import os

import numpy as np

KSTAGE = int(os.environ.get("KSTAGE", "99"))
WEAVE = int(os.environ.get("WEAVE", "1"))
PJPOOL = int(os.environ.get("PJPOOL", "0"))

B, N, C = 4, 4096, 512
NH, HD = 8, 64
M = 1024          # (64/2) * (64/2) spatial-reduced tokens
NHALF = 2048      # queries per core
LN_EPS = 1e-5

NQT = NHALF // 128
KT = C // 128     # 4 c-tiles
MT = M // 128     # 8 m-tiles
NCH = NHALF // 512

_cache = {}


def _build_nc():
    import concourse.tile as tile
    from concourse import bacc, mybir

    f32 = mybir.dt.float32
    f32r = mybir.dt.float32r
    bf16 = mybir.dt.bfloat16
    f8 = mybir.dt.float8e4
    AF = mybir.ActivationFunctionType
    OP = mybir.AluOpType
    PM = mybir.MatmulPerfMode

    # Pin Exp/Ln/Square to the one ACT table set that contains all three
    # (natural_log_exp_and_others); otherwise the set chooser alternates
    # between sets and pays a ~1.3us ACT_TABLE_LOAD per switch in the hot
    # loop.  Indices of the sets are preserved (walrus maps by index).
    import concourse.bacc as bacc_mod
    if not hasattr(bacc_mod, "_orig_get_activation_tables"):
        bacc_mod._orig_get_activation_tables = bacc_mod.get_activation_tables

        def _pinned_tables(arch):
            d = bacc_mod._orig_get_activation_tables(arch)
            strip = {AF.Exp, AF.Ln, AF.Square}
            return {
                name: (funcs if name == "natural_log_exp_and_others"
                       else funcs - strip)
                for name, funcs in d.items()
            }

        bacc_mod.get_activation_tables = _pinned_tables

    nc = bacc.Bacc("TRN2", target_bir_lowering=False, debug=False)

    xc_d = nc.dram_tensor("xc", [C, 4 * M], bf16, kind="ExternalInput")
    xq_d = nc.dram_tensor("xqT", [C, NHALF], bf16, kind="ExternalInput")
    qw_d = nc.dram_tensor("q_wT", [C, C], bf16, kind="ExternalInput")
    kw_d = nc.dram_tensor("k_wT", [C, C], bf16, kind="ExternalInput")
    vw_d = nc.dram_tensor("v_wT", [C, 2 * C], bf16, kind="ExternalInput")
    pw_d = nc.dram_tensor("p_wT", [C, C], bf16, kind="ExternalInput")
    chan_d = nc.dram_tensor("chan", [C, 8], f32, kind="ExternalInput")
    pb_d = nc.dram_tensor("p_b", [1, C], f32, kind="ExternalInput")
    out_d = nc.dram_tensor("out", [NHALF, C], bf16,
                           kind="ExternalOutput")

    with tile.TileContext(nc) as tc:
        _cms = {}

        def pool(name, bufs=1, space="SBUF", side=None):
            cm = tc.tile_pool(name=name, bufs=bufs, space=space, side=side)
            p = cm.__enter__()
            _cms[id(p)] = cm
            return p

        def close(*pools):
            for p in pools:
                _cms.pop(id(p)).__exit__(None, None, None)

        consts = pool("consts")
        ones_f = consts.tile([128, 128], f32, tag="ones_f", name="ones_f")
        nc.vector.memset(ones_f[:], 1.0)
        ones128 = consts.tile([128, 128], bf16, tag="ones128", name="ones128")
        nc.vector.tensor_copy(ones128[:], ones_f[:])
        onesPV = consts.tile([128, 1], bf16, tag="onesPV", name="onesPV")
        nc.vector.tensor_copy(onesPV[:], ones_f[:, 0:1])
        # [ones(64)|zeros(64)] / [zeros(64)|ones(64)] mask rows (all 128
        # partitions identical) for the normalize-broadcast matmuls (f32r)
        mask_f = consts.tile([128, 128], f32, tag="mask_f", name="mask_f")
        nc.vector.memset(mask_f[:], 1.0)
        nc.vector.memset(mask_f[:, 64:128], 0.0)
        maskAr = consts.tile([128, 128], f32r, tag="maskAr", name="maskAr")
        nc.vector.tensor_copy(maskAr[:], mask_f[:])
        nc.vector.memset(mask_f[:, 0:64], 0.0)
        nc.vector.memset(mask_f[:, 64:128], 1.0)
        maskBr = consts.tile([128, 128], f32r, tag="maskBr", name="maskBr")
        nc.vector.tensor_copy(maskBr[:], mask_f[:])
        zero_t = consts.tile([128, 1], f32, tag="zero_t", name="zero_t")
        nc.vector.memset(zero_t[:], 0.0)
        eps_t = consts.tile([128, 1], f32, tag="eps_t", name="eps_t")
        nc.vector.memset(eps_t[:], LN_EPS)
        # chan params: cols 0-3 conv taps, 4 sr_b
        chan = [consts.tile([128, 8], f32, tag=f"chan{k}", name=f"chan{k}")
                for k in range(KT)]
        pb_bc = consts.tile([128, C], f32, tag="pb", name="pb")
        warm = consts.tile([128, 1], f32, tag="warm", name="warm")
        nc.scalar.activation(warm[:], zero_t[:], AF.Exp, bias=zero_t[:])

        # =============== DMA issue (spread across engine queues) ==========
        # sync: xq (ch-major so Q proj can start after ~5 issues) + weights
        # pool: conv planes (k-major)   scalar: chan + pb broadcast
        xc_pool = pool("xcp", side="right")
        xc_sb = xc_pool.tile([128, KT * 4 * M], bf16, tag="xc", name="xc")
        xq_pool = pool("xqp", side="right")
        xq_sb = xq_pool.tile([128, KT * NHALF], bf16, tag="xq", name="xq")
        w_pool = pool("wp")
        qw = [w_pool.tile([128, C], bf16, tag=f"qw{k}", name=f"qw{k}")
              for k in range(KT)]
        kw = [w_pool.tile([128, C], bf16, tag=f"kw{k}", name=f"kw{k}")
              for k in range(KT)]
        vw = [w_pool.tile([128, 2 * C], bf16, tag=f"vw{k}", name=f"vw{k}")
              for k in range(KT)]
        pw = [w_pool.tile([128, C], bf16, tag=f"pw{k}", name=f"pw{k}")
              for k in range(KT)]

        for k in range(KT):
            nc.scalar.dma_start(chan[k][:],
                                chan_d.ap()[128 * k:128 * (k + 1), :])
        for k in range(KT):
            for t in range(0, 4, 2):
                nc.scalar.dma_start(
                    xc_sb[:, k * 4 * M + t * M:k * 4 * M + (t + 2) * M],
                    xc_d.ap()[128 * k:128 * (k + 1), t * M:(t + 2) * M])
        nc.scalar.dma_start(pb_bc[:], pb_d.ap().to_broadcast([128, C]))

        for k in range(KT):
            nc.sync.dma_start(
                xq_sb[:, k * NHALF:k * NHALF + 512],
                xq_d.ap()[128 * k:128 * (k + 1), 0:512])
        for k in range(KT):
            nc.sync.dma_start(qw[k][:], qw_d.ap()[128 * k:128 * (k + 1), :])
        for ch in range(1, NCH):
            for k in range(KT):
                nc.sync.dma_start(
                    xq_sb[:, k * NHALF + 512 * ch:k * NHALF + 512 * (ch + 1)],
                    xq_d.ap()[128 * k:128 * (k + 1), 512 * ch:512 * (ch + 1)])
        for k in range(KT):
            nc.sync.dma_start(kw[k][:], kw_d.ap()[128 * k:128 * (k + 1), :])
        for k in range(KT):
            nc.sync.dma_start(vw[k][:], vw_d.ap()[128 * k:128 * (k + 1), :])
        for k in range(KT):
            nc.sync.dma_start(pw[k][:], pw_d.ap()[128 * k:128 * (k + 1), :])


        # persistent SBUF pools (opened before the short-lived conv pools so
        # the pool stack stays LIFO)
        qt_pool = pool("qtp")
        qT = [qt_pool.tile([128, NHALF], bf16, tag=f"qT{k}", name=f"qT{k}")
              for k in range(KT)]
        zp = pool("zp")
        z = [zp.tile([128, M], bf16, tag=f"z{k}", name=f"z{k}")
             for k in range(KT)]
        kvp = pool("kvp")
        kTt = [kvp.tile([128, M], bf16, tag=f"kT{p}", name=f"kT{p}")
               for p in range(KT)]
        # vt[mt]: [m-tile, 2C] packed per pair p: cols 256p..+127 = [v_A|0],
        # cols 256p+128..+255 = [0|v_B]  (zero-padding baked into v_wT on
        # host).  NOTE: writing psum rows 64:128 via tile_position=(0,64)
        # column tiling hard-crashes the HW (quadrant bug, verified with a
        # minimal repro), so PV keeps the full-array zero-padded form.
        vt = [kvp.tile([128, 2 * C], bf16, tag=f"vt{m}", name=f"vt{m}")
              for m in range(MT)]

        # =============== phase C: depthwise conv (DVE, bf16 2x) ===========
        ytp = pool("ytp")
        yt = [ytp.tile([128, M], bf16, tag=f"yt{k}", name=f"yt{k}")
              for k in range(KT)]
        ysq = [ytp.tile([128, M], bf16, tag=f"ysq{k}", name=f"ysq{k}")
               for k in range(KT)]
        cv = pool("cv", bufs=2)
        for k in range(KT):
            acc = cv.tile([128, M], bf16, tag="cacc", name="cacc")
            base = k * 4 * M
            nc.vector.tensor_scalar_mul(
                acc[:], xc_sb[:, base:base + M], chan[k][:, 0:1])
            for t in range(1, 4):
                nc.vector.scalar_tensor_tensor(
                    acc[:], xc_sb[:, base + t * M:base + (t + 1) * M],
                    chan[k][:, t:t + 1], acc[:],
                    op0=OP.mult, op1=OP.add)
            nc.vector.tensor_scalar_add(yt[k][:], acc[:], chan[k][:, 4:5])
            # square on GpSimd: idle in the prologue, keeps DVE on the
            # conv critical path (conv_k must keep pace with the xc DMA)
            nc.gpsimd.tensor_tensor(ysq[k][:], yt[k][:], yt[k][:], op=OP.mult)

        # =============== phase Q: q^T (PE) + LN sums interleaved ==========
        q_psum = pool("q_ps", bufs=4, space="PSUM")
        st_psum = pool("st_ps", space="PSUM")
        SY = st_psum.tile([128, M], f32, tag="SY", name="SY")
        SY2 = st_psum.tile([128, M], f32, tag="SY2", name="SY2")
        for co in range(KT):
            for ch in range(NCH):
                ps = q_psum.tile([128, 512], f32, tag="q", name="q")
                for k in range(KT):
                    nc.tensor.matmul(
                        ps[:],
                        qw[k][:, 128 * co:128 * (co + 1)],
                        xq_sb[:, k * NHALF + 512 * ch:
                                 k * NHALF + 512 * (ch + 1)],
                        start=(k == 0), stop=(k == KT - 1),
                    )
                nc.scalar.copy(qT[co][:, 512 * ch:512 * (ch + 1)], ps[:])
            # LN sum matmuls for k-tile `co`, interleaved so the PE reaches
            # them right about when the conv (DVE) has produced yt/ysq.
            k = co
            for ch2 in range(M // 512):
                s_ = slice(512 * ch2, 512 * (ch2 + 1))
                nc.tensor.matmul(SY[:, s_], ones128[:], yt[k][:, s_],
                                 start=(k == 0), stop=(k == KT - 1))
                nc.tensor.matmul(SY2[:, s_], ones128[:], ysq[k][:, s_],
                                 start=(k == 0), stop=(k == KT - 1))

        # =============== LN stats + normalize (z = (y-mu)*inv_std) ========
        mu2 = cv.tile([128, M], f32, tag="stat", name="mu2")
        nc.scalar.activation(mu2[:], SY[:], AF.Square, bias=zero_t[:],
                             scale=1.0 / C)
        var = cv.tile([128, M], f32, tag="stat", name="var")
        nc.vector.scalar_tensor_tensor(
            var[:], SY2[:], 1.0 / C, mu2[:], op0=OP.mult, op1=OP.subtract)
        lgv = cv.tile([128, M], f32, tag="stat", name="lgv")
        nc.scalar.activation(lgv[:], var[:], AF.Ln, bias=eps_t[:])
        invb = cv.tile([128, M], bf16, tag="invb", name="invb")
        nc.scalar.activation(invb[:], lgv[:], AF.Exp, bias=zero_t[:],
                             scale=-0.5)
        mu_sb = cv.tile([128, M], bf16, tag="mu", name="mu_sb")
        nc.vector.tensor_scalar_mul(mu_sb[:], SY[:], 1.0 / C)
        for k in range(KT):
            t1 = cv.tile([128, M], bf16, tag="t1", name="t1")
            nc.vector.tensor_tensor(t1[:], yt[k][:], mu_sb[:], op=OP.subtract)
            nc.vector.tensor_tensor(z[k][:], t1[:], invb[:], op=OP.mult)
        close(st_psum, q_psum, cv, ytp, xq_pool, xc_pool)

        # =============== attention-era PSUM pools ==========================
        # Explicit bank budget (8 banks total):
        #   S:  QK scores, 2 bufs x [128,1024] f32             = 4 banks
        #   O:  PV accumulators, 2 bufs x [128,512]            = 2 banks
        #   sb: softmax-sum rows + normalize-broadcast, 1 buf  = 1 bank
        #       (s_t and bc rotate through the single slot; WAR deps
        #        serialize them, which the slot-1/2/3 action schedule
        #        keeps off the critical path)
        #   pj: out-projection chains, 1 buf x [128,512]       = 1 bank
        # The woven K/V-proj chains (groups 0-2 only) borrow the pj and
        # sb slots, which have no other use during those windows.
        S_psum = pool("S_ps", bufs=2, space="PSUM")
        O_psum = pool("O_ps", bufs=2, space="PSUM")
        sb_psum = pool("sb_ps", bufs=(1 if PJPOOL else 2), space="PSUM")
        pj_psum = pool("pj_ps", bufs=1, space="PSUM") if PJPOOL else None

        def chain_tile(which):
            # psum scratch for a woven K/V-proj chain: alternate the pj and
            # sb slots so consecutive chains overlap MM+drain two-deep.
            if PJPOOL and which % 2 == 0:
                return pj_psum.tile([128, 512], f32, tag="pj", name="cps")
            return sb_psum.tile([128, 512], f32, tag="sb", name="cps")

        def emit_vchain(mt, ps_fn=None):
            for vh in range(2):
                vs = slice(512 * vh, 512 * (vh + 1))
                vps = ps_fn(vh) if ps_fn is not None else \
                    S_psum.tile([128, 512], f32, tag="S", name="vps")
                for k in range(KT):
                    nc.tensor.matmul(
                        vps[:], z[k][:, 128 * mt:128 * (mt + 1)],
                        vw[k][:, vs],
                        start=(k == 0), stop=(k == KT - 1))
                nc.vector.tensor_copy(vt[mt][:, vs], vps[:])

        def emit_kchain(p, ps_fn=None):
            for ch2 in range(M // 512):
                s_ = slice(512 * ch2, 512 * (ch2 + 1))
                kps = ps_fn(ch2) if ps_fn is not None else \
                    S_psum.tile([128, 512], f32, tag="S", name="kps")
                for k in range(KT):
                    nc.tensor.matmul(
                        kps[:], kw[k][:, 128 * p:128 * (p + 1)],
                        z[k][:, s_],
                        start=(k == 0), stop=(k == KT - 1))
                nc.vector.tensor_copy(kTt[p][:, s_], kps[:])

        if KSTAGE >= 1:
            emit_kchain(0)
            emit_vchain(0)
            emit_vchain(1)
            if not WEAVE:
                for p_ in range(1, KT):
                    emit_kchain(p_)
                for mt_ in range(2, MT):
                    emit_vchain(mt_)

        # =============== phase A: attention, ch outer / pair inner ========
        OT_pool = pool("OTp", side="right")
        OT = [OT_pool.tile([128, NHALF], bf16, tag=f"OT{p}", name=f"OT{p}")
              for p in range(KT)]
        ss_pool = pool("ssp", side="right", bufs=2)
        sinv_pool = pool("sinvp", bufs=2)
        ppool = pool("ptile", bufs=4)
        opool = pool("outp", bufs=3)

        pending_pv = None

        def emit_pv(pv):
            pp, mt, pt, o_ps = pv
            nc.tensor.matmul(
                o_ps[:], vt[mt][:, 256 * pp:256 * pp + 128],
                pt[:, 0:512],
                start=(mt == 0), stop=False)
            nc.tensor.matmul(
                o_ps[:], vt[mt][:, 256 * pp + 128:256 * (pp + 1)],
                pt[:, 512:1024],
                start=False, stop=(mt == MT - 1))

        def emit_smm(st):
            # softmax-sum chains for a finished chunk: head A -> partition 0,
            # head B -> partition 32 of one PSUM bank (so the DVE reciprocal
            # runs lane-parallel and feeds the broadcast matmuls directly).
            accE, accO = st["accE"], st["accO"]
            nc.vector.tensor_tensor(accE[:], accE[:], accO[:], op=OP.add)
            s_t = sb_psum.tile([128, 512], f32, tag="sb", name="s_t")
            # head A sum broadcast to rows 0..32 (M=33 ones) so the single
            # contiguous reciprocal read [0:33] never touches bytes last
            # written by another slot tenant (the woven K/V chains share
            # this bank); row 32 then overwritten with head B's sum.
            nc.tensor.matmul(s_t[0:33, :], ones128[:, 0:33], accE[:, 0:512],
                             start=True, stop=True)
            nc.tensor.matmul(s_t[32:33, :], onesPV[:], accE[:, 512:1024],
                             start=True, stop=True, skip_group_check=True)
            st["s_t"] = s_t

        def emit_recip(st):
            # one contiguous call covering partitions 0..32 (rows 1-31 are
            # never-read garbage; single-row calls at partition offset 32
            # mis-evaluate, and strided partition APs fail codegen)
            sinv = sinv_pool.tile([33, 512], f32, tag="sinv", name="sinv")
            nc.vector.reciprocal_approx_fast(sinv[0:33, :],
                                             st["s_t"][0:33, :])
            sinr = sinv_pool.tile([33, 512], f32r, tag="sinr", name="sinr")
            nc.vector.tensor_copy(sinr[0:1, :], sinv[0:1, :])
            nc.vector.tensor_copy(sinr[32:33, :], sinv[32:33, :])
            st["sinr"] = sinr

        def emit_bc(st):
            sinr = st["sinr"]
            bc = sb_psum.tile([128, 512], f32, tag="sb", name="bc")
            # Proper open/close pairing: MM1 opens (rows 0:64 = sinv_A,
            # 64:128 = 0), MM2 accumulates sinv_B and closes.  No
            # skip_group_check: the sim/dep group state must see the close,
            # else readers observe an open group.
            nc.tensor.matmul(bc[:], maskAr[0:1, :], sinr[0:1, :],
                             start=True, stop=False)
            nc.tensor.matmul(bc[:], maskBr[32:33, :], sinr[32:33, :],
                             start=False, stop=True)
            st["bc"] = bc

        def emit_ot(st):
            p, ch = st["p"], st["ch"]
            nsl = slice(512 * ch, 512 * (ch + 1))
            nc.vector.tensor_copy(OT[p][:, nsl], st["o_ps"][:])
            nc.vector.tensor_tensor(OT[p][:, nsl], OT[p][:, nsl],
                                    st["bc"][:], op=OP.mult)

        def emit_pj_nt(nt, tail=False):
            # output projection + bias + store for one 128-query n-tile.
            # Steady state uses the dedicated 1-bank pj slot (drained well
            # within one mt period); the end-of-kernel tail alternates the
            # freed S slots so the last 4 chains pipeline two-deep.
            if tail or not PJPOOL:
                pj = S_psum.tile([128, 512], f32, tag="S", name="pj")
            else:
                pj = pj_psum.tile([128, 512], f32, tag="pj", name="pj")
            for p2 in range(KT):
                nc.tensor.matmul(
                    pj[:], OT[p2][:, 128 * nt:128 * (nt + 1)],
                    pw[p2][:],
                    start=(p2 == 0), stop=(p2 == KT - 1))
            ob = opool.tile([128, 512], bf16, tag="ob", name="ob")
            nc.vector.tensor_tensor(ob[:], pj[:], pb_bc[:],
                                    op=OP.add)
            nc.sync.dma_start(out_d.ap()[128 * nt:128 * (nt + 1), :],
                              ob[:])

        actions = {}     # mt -> list of thunks, for the current chunk
        nweave = 0       # round-robin counter for woven-chain psum slots
        n_ch = 0 if KSTAGE < 2 else min(KSTAGE - 1, NCH)
        for ch in range(n_ch):
            for p in range(KT):
                st = {"p": p, "ch": ch,
                      "o_ps": O_psum.tile([128, 512], f32, tag="o",
                                          name="o_ps"),
                      "accE": ss_pool.tile([128, 1024], bf16, tag="accE",
                                           name="accE"),
                      "accO": ss_pool.tile([128, 1024], bf16, tag="accO",
                                           name="accO")}
                nsl = slice(512 * ch, 512 * (ch + 1))
                for mt in range(MT):
                    for fn in actions.pop(mt, ()):
                        fn()
                    msl = slice(128 * mt, 128 * (mt + 1))
                    S_ps = S_psum.tile([128, 1024], f32, tag="S",
                                       name="S_ps")
                    nc.tensor.matmul(
                        S_ps[:, 0:512],
                        kTt[p][0:64, msl], qT[p][0:64, nsl],
                        start=True, stop=True, tile_position=(0, 0))
                    nc.tensor.matmul(
                        S_ps[:, 512:1024],
                        kTt[p][64:128, msl], qT[p][64:128, nsl],
                        start=True, stop=True, tile_position=(64, 0))
                    pt = ppool.tile([128, 1024], bf16, tag="pt", name="pt")
                    nc.scalar.activation(pt[:], S_ps[:], AF.Exp,
                                         bias=zero_t[:])
                    if pending_pv is not None:
                        emit_pv(pending_pv)
                    pending_pv = (p, mt, pt, st["o_ps"])
                    if mt == 0:
                        nc.vector.tensor_copy(st["accE"][:], pt[:])
                    elif mt == 1:
                        nc.vector.tensor_copy(st["accO"][:], pt[:])
                    elif mt % 2 == 0:
                        nc.gpsimd.tensor_tensor(st["accE"][:], st["accE"][:],
                                                pt[:], op=OP.add)
                    else:
                        nc.vector.tensor_tensor(st["accO"][:], st["accO"][:],
                                                pt[:], op=OP.add)
                    if WEAVE and ch == 0 and p == 0 and mt < 6:
                        emit_vchain(mt + 2,
                                    ps_fn=lambda vh, b=nweave:
                                    chain_tile(b + vh))
                        nweave += 2
                    if WEAVE and ch == 0 and p < KT - 1 and mt == 5:
                        emit_kchain(p + 1,
                                    ps_fn=lambda c2, b=nweave:
                                    chain_tile(b + c2))
                        nweave += 2
                # Epilogue deferred into the next group's mt stream, one
                # stage per slot so no PE stall and no psum-slot pile-up:
                # smm+recip@1, bc@2, OT@3, pj@4..7 (pj only after the
                # chunk's last pair).
                actions = {
                    1: [lambda st=st: emit_smm(st),
                        lambda st=st: emit_recip(st)],
                    2: [lambda st=st: emit_bc(st)],
                    3: [lambda st=st: emit_ot(st)],
                }
                if p == KT - 1:
                    tail = (ch == n_ch - 1)
                    for i, nt in enumerate(range(4 * ch, 4 * ch + 4)):
                        actions[4 + i] = [
                            lambda nt=nt, tail=tail: emit_pj_nt(nt, tail)]
        # drain: last chunk's PV + epilogue + all projection groups
        if pending_pv is not None:
            emit_pv(pending_pv)
        for mt in sorted(actions):
            for fn in actions[mt]:
                fn()
        close(opool, ppool, sinv_pool,
              *([pj_psum] if PJPOOL else []), sb_psum, O_psum, S_psum,
              ss_pool, OT_pool,
              kvp, zp, qt_pool, w_pool, consts)

    nc.compile()
    return nc


def _get_nc():
    if "nc" not in _cache:
        _cache["nc"] = _build_nc()
    return _cache["nc"]


def _make_in_maps(x, q_w, kv_w, proj_w, proj_b, sr_w, sr_b, ln_g, ln_b):
    import ml_dtypes
    bf = ml_dtypes.bfloat16

    x = np.asarray(x, np.float32)
    q_w = np.asarray(q_w, np.float32)
    kv_w = np.asarray(kv_w, np.float32)
    proj_w = np.asarray(proj_w, np.float32)
    g = np.asarray(ln_g, np.float32)
    lb = np.asarray(ln_b, np.float32)
    scale = HD ** -0.5

    q_wT = np.ascontiguousarray((q_w.T * scale).astype(bf))
    # fold ln_g into the kv weight columns (k = k_w @ (z*g + ln_b))
    k_wT = np.ascontiguousarray((kv_w[:C] * g[None, :]).T.astype(bf))
    v_wg = kv_w[C:] * g[None, :]
    v_wT_raw = v_wg.T  # [c_in, c_out=h*64+d]
    # packed per head pair p: [v_A|0] then [0|v_B] (for PV psum packing)
    v_wT = np.zeros((C, 2 * C), np.float32)
    for p in range(KT):
        v_wT[:, 256 * p:256 * p + 64] = v_wT_raw[:, 128 * p:128 * p + 64]
        v_wT[:, 256 * p + 192:256 * p + 256] = \
            v_wT_raw[:, 128 * p + 64:128 * p + 128]
    v_wT = np.ascontiguousarray(v_wT.astype(bf))
    p_wT = np.ascontiguousarray(proj_w.T.astype(bf))
    chan = np.zeros((C, 8), np.float32)
    sr_w = np.asarray(sr_w, np.float32)
    for di in range(2):
        for dj in range(2):
            chan[:, di * 2 + dj] = sr_w[:, 0, di, dj]
    chan[:, 4] = np.asarray(sr_b, np.float32)
    # ln_b folds: K-side bias is killed by softmax shift-invariance; the
    # V-side bias contributes exactly v_w@ln_b per channel (sum(P)/s == 1),
    # which commutes through the projection into an output bias.
    vb = kv_w[C:] @ lb
    p_b = (np.asarray(proj_b, np.float32) + proj_w @ vb).reshape(1, C)

    in_maps = []
    for d in range(8):
        b, half = d // 2, d % 2
        xbT = x[b].T  # [C, N]
        xCHW = xbT.reshape(C, 64, 64)
        planes = np.stack([xCHW[:, di::2, dj::2].reshape(C, M)
                           for di in range(2) for dj in range(2)], axis=1)
        xc = np.ascontiguousarray(planes.reshape(C, 4 * M).astype(bf))
        in_maps.append({
            "xc": xc,
            "xqT": np.ascontiguousarray(
                xbT[:, half * NHALF:(half + 1) * NHALF].astype(bf)),
            "q_wT": q_wT, "k_wT": k_wT, "v_wT": v_wT, "p_wT": p_wT,
            "chan": chan, "p_b": p_b,
        })
    return in_maps


def _assemble(res):
    out = np.empty((B, N, C), np.float32)
    for d in range(8):
        b, half = d // 2, d % 2
        out[b, half * NHALF:(half + 1) * NHALF, :] = res.results[d]["out"]
    return out


def kernel(x, q_w, kv_w, proj_w, proj_b, sr_w, sr_b, ln_g, ln_b, H, W):
    from concourse.bass_utils import run_bass_kernel_spmd

    nc = _get_nc()
    in_maps = _make_in_maps(x, q_w, kv_w, proj_w, proj_b, sr_w, sr_b,
                            ln_g, ln_b)
    res = run_bass_kernel_spmd(nc, in_maps, core_ids=list(range(8)))
    return _assemble(res)



# revision 32
# speedup vs baseline: 1.0250x; 1.0097x over previous
"""Trainium2 Bass kernel for PVT-style spatial-reduction attention.

Reference computation (B=4, N=4096, C=512, 8 heads, head_dim=64):
  q = (x @ q_w.T) * hd**-0.5                    -> [B, N, C]
  x_ = depthwise_conv2x2_stride2(x as NCHW 64x64) + sr_b -> [B, M=1024, C]
  x_ = layernorm(x_) * ln_g + ln_b
  k, v = split(x_ @ kv_w.T)                      -> [B, nh, M, hd] each
  out = softmax(q k^T) v                         -> [B, N, C]
  out = out @ proj_w.T + proj_b

Sharding: 8 cores, core d handles batch b=d//2, query-half h=d%2 (2048
queries).  Each core computes its output slice independently (the small KV
path is recomputed per half); the host concatenates.  No collectives.

Host-side folds: ln_g is folded into k_w/v_w columns; ln_b folds exactly
into proj_b (softmax shift-invariance kills the K-side bias, and the
V-side bias times sum(P)/s == v_w@ln_b is constant per channel, which
commutes through the projection).  The conv input arrives pre-shuffled
into 4 stride-2 tap planes so the depthwise conv is 4 contiguous
multiply-accumulates per c-tile (bf16, DVE 2x mode).

Engine plan: DVE runs conv + LN-normalize + KV psum drains + softmax-sum
accumulation (shared with GpSimd) + 1/s (reciprocal_approx_fast, spread
to partitions 0/32 by the ones-matmul sum chains) + epilogue multiplies.
ACT runs only q^T psum drains, LN stats, and the per-mt Exp.  PE order:
Q proj (interleaved with LN sum matmuls), K proj (all pairs), V proj,
then attention ch-outer/pair-inner with the output projection for each
512-query column group interleaved right after its last pair finishes.
Chunk epilogues (sum matmuls, reciprocal, normalize-broadcast matmuls,
OT write) are deferred into the next chunk's mt stream so no PE stall
sits on the critical path.
"""

import os

import numpy as np

KSTAGE = int(os.environ.get("KSTAGE", "99"))
WEAVE = int(os.environ.get("WEAVE", "1"))

B, N, C = 4, 4096, 512
NH, HD = 8, 64
M = 1024          # (64/2) * (64/2) spatial-reduced tokens
NHALF = 2048      # queries per core
LN_EPS = 1e-5

NQT = NHALF // 128
KT = C // 128     # 4 c-tiles
MT = M // 128     # 8 m-tiles
NCH = NHALF // 512

_cache = {}


def _build_nc():
    import concourse.tile as tile
    from concourse import bacc, mybir

    f32 = mybir.dt.float32
    f32r = mybir.dt.float32r
    bf16 = mybir.dt.bfloat16
    f8 = mybir.dt.float8e4
    AF = mybir.ActivationFunctionType
    OP = mybir.AluOpType
    PM = mybir.MatmulPerfMode

    # Pin Exp/Ln/Square to the one ACT table set that contains all three
    # (natural_log_exp_and_others); otherwise the set chooser alternates
    # between sets and pays a ~1.3us ACT_TABLE_LOAD per switch in the hot
    # loop.  Indices of the sets are preserved (walrus maps by index).
    import concourse.bacc as bacc_mod
    if not hasattr(bacc_mod, "_orig_get_activation_tables"):
        bacc_mod._orig_get_activation_tables = bacc_mod.get_activation_tables

        def _pinned_tables(arch):
            d = bacc_mod._orig_get_activation_tables(arch)
            strip = {AF.Exp, AF.Ln, AF.Square}
            return {
                name: (funcs if name == "natural_log_exp_and_others"
                       else funcs - strip)
                for name, funcs in d.items()
            }

        bacc_mod.get_activation_tables = _pinned_tables

    nc = bacc.Bacc("TRN2", target_bir_lowering=False, debug=False)

    xc_d = nc.dram_tensor("xc", [C, 4 * M], bf16, kind="ExternalInput")
    xq_d = nc.dram_tensor("xqT", [C, NHALF], bf16, kind="ExternalInput")
    qw_d = nc.dram_tensor("q_wT", [C, C], bf16, kind="ExternalInput")
    kw_d = nc.dram_tensor("k_wT", [C, C], bf16, kind="ExternalInput")
    vw_d = nc.dram_tensor("v_wT", [C, 2 * C], bf16, kind="ExternalInput")
    pw_d = nc.dram_tensor("p_wT", [C, C], bf16, kind="ExternalInput")
    chan_d = nc.dram_tensor("chan", [C, 8], f32, kind="ExternalInput")
    pb_d = nc.dram_tensor("p_b", [1, C], f32, kind="ExternalInput")
    out_d = nc.dram_tensor("out", [NHALF, C], bf16,
                           kind="ExternalOutput")

    with tile.TileContext(nc) as tc:
        _cms = {}

        def pool(name, bufs=1, space="SBUF", side=None):
            cm = tc.tile_pool(name=name, bufs=bufs, space=space, side=side)
            p = cm.__enter__()
            _cms[id(p)] = cm
            return p

        def close(*pools):
            for p in pools:
                _cms.pop(id(p)).__exit__(None, None, None)

        consts = pool("consts")
        ones_f = consts.tile([128, 128], f32, tag="ones_f", name="ones_f")
        nc.vector.memset(ones_f[:], 1.0)
        ones128 = consts.tile([128, 128], bf16, tag="ones128", name="ones128")
        nc.vector.tensor_copy(ones128[:], ones_f[:])
        onesPV = consts.tile([128, 1], bf16, tag="onesPV", name="onesPV")
        nc.vector.tensor_copy(onesPV[:], ones_f[:, 0:1])
        # [ones(64)|zeros(64)] / [zeros(64)|ones(64)] mask rows (all 128
        # partitions identical) for the normalize-broadcast matmuls (f32r)
        mask_f = consts.tile([128, 128], f32, tag="mask_f", name="mask_f")
        nc.vector.memset(mask_f[:], 1.0)
        nc.vector.memset(mask_f[:, 64:128], 0.0)
        maskAr = consts.tile([128, 128], f32r, tag="maskAr", name="maskAr")
        nc.vector.tensor_copy(maskAr[:], mask_f[:])
        nc.vector.memset(mask_f[:, 0:64], 0.0)
        nc.vector.memset(mask_f[:, 64:128], 1.0)
        maskBr = consts.tile([128, 128], f32r, tag="maskBr", name="maskBr")
        nc.vector.tensor_copy(maskBr[:], mask_f[:])
        zero_t = consts.tile([128, 1], f32, tag="zero_t", name="zero_t")
        nc.vector.memset(zero_t[:], 0.0)
        eps_t = consts.tile([128, 1], f32, tag="eps_t", name="eps_t")
        nc.vector.memset(eps_t[:], LN_EPS)
        # chan params: cols 0-3 conv taps, 4 sr_b
        chan = [consts.tile([128, 8], f32, tag=f"chan{k}", name=f"chan{k}")
                for k in range(KT)]
        pb_bc = consts.tile([128, C], f32, tag="pb", name="pb")
        warm = consts.tile([128, 1], f32, tag="warm", name="warm")
        nc.scalar.activation(warm[:], zero_t[:], AF.Exp, bias=zero_t[:])

        # =============== DMA issue (spread across engine queues) ==========
        # sync: xq (ch-major so Q proj can start after ~5 issues) + weights
        # pool: conv planes (k-major)   scalar: chan + pb broadcast
        xc_pool = pool("xcp", side="right")
        xc_sb = xc_pool.tile([128, KT * 4 * M], bf16, tag="xc", name="xc")
        xq_pool = pool("xqp", side="right")
        xq_sb = xq_pool.tile([128, KT * NHALF], bf16, tag="xq", name="xq")
        w_pool = pool("wp")
        qw = [w_pool.tile([128, C], bf16, tag=f"qw{k}", name=f"qw{k}")
              for k in range(KT)]
        kw = [w_pool.tile([128, C], bf16, tag=f"kw{k}", name=f"kw{k}")
              for k in range(KT)]
        vw = [w_pool.tile([128, 2 * C], bf16, tag=f"vw{k}", name=f"vw{k}")
              for k in range(KT)]
        pw = [w_pool.tile([128, C], bf16, tag=f"pw{k}", name=f"pw{k}")
              for k in range(KT)]

        for k in range(KT):
            nc.scalar.dma_start(chan[k][:],
                                chan_d.ap()[128 * k:128 * (k + 1), :])
        for k in range(KT):
            for t in range(0, 4, 2):
                nc.scalar.dma_start(
                    xc_sb[:, k * 4 * M + t * M:k * 4 * M + (t + 2) * M],
                    xc_d.ap()[128 * k:128 * (k + 1), t * M:(t + 2) * M])
        nc.scalar.dma_start(pb_bc[:], pb_d.ap().to_broadcast([128, C]))

        for k in range(KT):
            nc.sync.dma_start(
                xq_sb[:, k * NHALF:k * NHALF + 512],
                xq_d.ap()[128 * k:128 * (k + 1), 0:512])
        for k in range(KT):
            nc.sync.dma_start(qw[k][:], qw_d.ap()[128 * k:128 * (k + 1), :])
        for ch in range(1, NCH):
            for k in range(KT):
                nc.sync.dma_start(
                    xq_sb[:, k * NHALF + 512 * ch:k * NHALF + 512 * (ch + 1)],
                    xq_d.ap()[128 * k:128 * (k + 1), 512 * ch:512 * (ch + 1)])
        for k in range(KT):
            nc.sync.dma_start(kw[k][:], kw_d.ap()[128 * k:128 * (k + 1), :])
        for k in range(KT):
            nc.sync.dma_start(vw[k][:], vw_d.ap()[128 * k:128 * (k + 1), :])
        for k in range(KT):
            nc.sync.dma_start(pw[k][:], pw_d.ap()[128 * k:128 * (k + 1), :])


        # persistent SBUF pools (opened before the short-lived conv pools so
        # the pool stack stays LIFO)
        qt_pool = pool("qtp")
        qT = [qt_pool.tile([128, NHALF], bf16, tag=f"qT{k}", name=f"qT{k}")
              for k in range(KT)]
        zp = pool("zp")
        z = [zp.tile([128, M], bf16, tag=f"z{k}", name=f"z{k}")
             for k in range(KT)]
        kvp = pool("kvp")
        kTt = [kvp.tile([128, M], bf16, tag=f"kT{p}", name=f"kT{p}")
               for p in range(KT)]
        # vt[mt]: [m-tile, 2C] packed per pair p: cols 256p..+127 = [v_A|0],
        # cols 256p+128..+255 = [0|v_B]  (zero-padding baked into v_wT on
        # host).  NOTE: writing psum rows 64:128 via tile_position=(0,64)
        # column tiling hard-crashes the HW (quadrant bug, verified with a
        # minimal repro), so PV keeps the full-array zero-padded form.
        vt = [kvp.tile([128, 2 * C], bf16, tag=f"vt{m}", name=f"vt{m}")
              for m in range(MT)]

        # =============== phase C: depthwise conv (DVE, bf16 2x) ===========
        ytp = pool("ytp")
        yt = [ytp.tile([128, M], bf16, tag=f"yt{k}", name=f"yt{k}")
              for k in range(KT)]
        ysq = [ytp.tile([128, M], bf16, tag=f"ysq{k}", name=f"ysq{k}")
               for k in range(KT)]
        cv = pool("cv", bufs=2)
        for k in range(KT):
            acc = cv.tile([128, M], bf16, tag="cacc", name="cacc")
            base = k * 4 * M
            nc.vector.tensor_scalar_mul(
                acc[:], xc_sb[:, base:base + M], chan[k][:, 0:1])
            for t in range(1, 4):
                nc.vector.scalar_tensor_tensor(
                    acc[:], xc_sb[:, base + t * M:base + (t + 1) * M],
                    chan[k][:, t:t + 1], acc[:],
                    op0=OP.mult, op1=OP.add)
            nc.vector.tensor_scalar_add(yt[k][:], acc[:], chan[k][:, 4:5])
            # square on GpSimd: idle in the prologue, keeps DVE on the
            # conv critical path (conv_k must keep pace with the xc DMA)
            nc.gpsimd.tensor_tensor(ysq[k][:], yt[k][:], yt[k][:], op=OP.mult)

        # =============== phase Q: q^T (PE) + LN sums interleaved ==========
        q_psum = pool("q_ps", bufs=4, space="PSUM")
        st_psum = pool("st_ps", space="PSUM")
        SY = st_psum.tile([128, M], f32, tag="SY", name="SY")
        SY2 = st_psum.tile([128, M], f32, tag="SY2", name="SY2")
        for co in range(KT):
            for ch in range(NCH):
                ps = q_psum.tile([128, 512], f32, tag="q", name="q")
                for k in range(KT):
                    nc.tensor.matmul(
                        ps[:],
                        qw[k][:, 128 * co:128 * (co + 1)],
                        xq_sb[:, k * NHALF + 512 * ch:
                                 k * NHALF + 512 * (ch + 1)],
                        start=(k == 0), stop=(k == KT - 1),
                    )
                nc.scalar.copy(qT[co][:, 512 * ch:512 * (ch + 1)], ps[:])
            # LN sum matmuls for k-tile `co`, interleaved so the PE reaches
            # them right about when the conv (DVE) has produced yt/ysq.
            k = co
            for ch2 in range(M // 512):
                s_ = slice(512 * ch2, 512 * (ch2 + 1))
                nc.tensor.matmul(SY[:, s_], ones128[:], yt[k][:, s_],
                                 start=(k == 0), stop=(k == KT - 1))
                nc.tensor.matmul(SY2[:, s_], ones128[:], ysq[k][:, s_],
                                 start=(k == 0), stop=(k == KT - 1))

        # =============== LN stats + normalize (z = (y-mu)*inv_std) ========
        mu2 = cv.tile([128, M], f32, tag="stat", name="mu2")
        nc.scalar.activation(mu2[:], SY[:], AF.Square, bias=zero_t[:],
                             scale=1.0 / C)
        var = cv.tile([128, M], f32, tag="stat", name="var")
        nc.vector.scalar_tensor_tensor(
            var[:], SY2[:], 1.0 / C, mu2[:], op0=OP.mult, op1=OP.subtract)
        lgv = cv.tile([128, M], f32, tag="stat", name="lgv")
        nc.scalar.activation(lgv[:], var[:], AF.Ln, bias=eps_t[:])
        invb = cv.tile([128, M], bf16, tag="invb", name="invb")
        nc.scalar.activation(invb[:], lgv[:], AF.Exp, bias=zero_t[:],
                             scale=-0.5)
        mu_sb = cv.tile([128, M], bf16, tag="mu", name="mu_sb")
        nc.vector.tensor_scalar_mul(mu_sb[:], SY[:], 1.0 / C)
        for k in range(KT):
            t1 = cv.tile([128, M], bf16, tag="t1", name="t1")
            nc.vector.tensor_tensor(t1[:], yt[k][:], mu_sb[:], op=OP.subtract)
            nc.vector.tensor_tensor(z[k][:], t1[:], invb[:], op=OP.mult)
        close(st_psum, q_psum, cv, ytp, xq_pool, xc_pool)

        # =============== attention-era PSUM pools ==========================
        # Explicit bank budget (8 banks total):
        #   S:  QK scores, 2 bufs x [128,1024] f32             = 4 banks
        #   O:  PV accumulators, 2 bufs x [128,512]            = 2 banks
        #   sb: softmax-sum rows + normalize-broadcast, 1 buf  = 1 bank
        #       (s_t and bc rotate through the single slot; WAR deps
        #        serialize them, which the slot-1/2/3 action schedule
        #        keeps off the critical path)
        #   pj: out-projection chains, 1 buf x [128,512]       = 1 bank
        # The woven K/V-proj chains (groups 0-2 only) borrow the pj and
        # sb slots, which have no other use during those windows.
        S_psum = pool("S_ps", bufs=2, space="PSUM")
        O_psum = pool("O_ps", bufs=2, space="PSUM")
        sb_psum = pool("sb_ps", bufs=2, space="PSUM")

        def chain_tile(which):
            # psum scratch for a woven K/V-proj chain: the sb pool's two
            # rotating slots pipeline consecutive chains (MM+drain) 2-deep.
            return sb_psum.tile([128, 512], f32, tag="sb", name="cps")

        def emit_vchain(mt, ps_fn=None):
            for vh in range(2):
                vs = slice(512 * vh, 512 * (vh + 1))
                vps = ps_fn(vh) if ps_fn is not None else \
                    S_psum.tile([128, 512], f32, tag="S", name="vps")
                for k in range(KT):
                    nc.tensor.matmul(
                        vps[:], z[k][:, 128 * mt:128 * (mt + 1)],
                        vw[k][:, vs],
                        start=(k == 0), stop=(k == KT - 1))
                nc.vector.tensor_copy(vt[mt][:, vs], vps[:])

        def emit_kchain(p, ps_fn=None):
            for ch2 in range(M // 512):
                s_ = slice(512 * ch2, 512 * (ch2 + 1))
                kps = ps_fn(ch2) if ps_fn is not None else \
                    S_psum.tile([128, 512], f32, tag="S", name="kps")
                for k in range(KT):
                    nc.tensor.matmul(
                        kps[:], kw[k][:, 128 * p:128 * (p + 1)],
                        z[k][:, s_],
                        start=(k == 0), stop=(k == KT - 1))
                nc.vector.tensor_copy(kTt[p][:, s_], kps[:])

        if KSTAGE >= 1:
            emit_kchain(0)
            emit_vchain(0)
            emit_vchain(1)
            if not WEAVE:
                for p_ in range(1, KT):
                    emit_kchain(p_)
                for mt_ in range(2, MT):
                    emit_vchain(mt_)

        # =============== phase A: attention, ch outer / pair inner ========
        OT_pool = pool("OTp", side="right")
        OT = [OT_pool.tile([128, NHALF], bf16, tag=f"OT{p}", name=f"OT{p}")
              for p in range(KT)]
        ss_pool = pool("ssp", side="right", bufs=2)
        sinv_pool = pool("sinvp", bufs=2)
        ppool = pool("ptile", bufs=4)
        opool = pool("outp", bufs=3)

        pending_pv = None

        def emit_pv(pv):
            pp, mt, pt, o_ps = pv
            nc.tensor.matmul(
                o_ps[:], vt[mt][:, 256 * pp:256 * pp + 128],
                pt[:, 0:512],
                start=(mt == 0), stop=False)
            nc.tensor.matmul(
                o_ps[:], vt[mt][:, 256 * pp + 128:256 * (pp + 1)],
                pt[:, 512:1024],
                start=False, stop=(mt == MT - 1))

        def emit_smm(st):
            # softmax-sum chains for a finished chunk: head A -> partition 0,
            # head B -> partition 32 of one PSUM bank (so the DVE reciprocal
            # runs lane-parallel and feeds the broadcast matmuls directly).
            accE, accO = st["accE"], st["accO"]
            nc.vector.tensor_tensor(accE[:], accE[:], accO[:], op=OP.add)
            s_t = sb_psum.tile([128, 512], f32, tag="sb", name="s_t")
            # head A sum broadcast to rows 0..32 (M=33 ones) so the single
            # contiguous reciprocal read [0:33] never touches bytes last
            # written by another slot tenant (the woven K/V chains share
            # this bank); row 32 then overwritten with head B's sum.
            nc.tensor.matmul(s_t[0:33, :], ones128[:, 0:33], accE[:, 0:512],
                             start=True, stop=True)
            nc.tensor.matmul(s_t[32:33, :], onesPV[:], accE[:, 512:1024],
                             start=True, stop=True, skip_group_check=True)
            st["s_t"] = s_t

        def emit_recip(st):
            # one contiguous call covering partitions 0..32 (rows 1-31 are
            # never-read garbage; single-row calls at partition offset 32
            # mis-evaluate, and strided partition APs fail codegen)
            sinv = sinv_pool.tile([33, 512], f32, tag="sinv", name="sinv")
            nc.vector.reciprocal_approx_fast(sinv[0:33, :],
                                             st["s_t"][0:33, :])
            sinr = sinv_pool.tile([33, 512], f32r, tag="sinr", name="sinr")
            nc.vector.tensor_copy(sinr[0:1, :], sinv[0:1, :])
            nc.vector.tensor_copy(sinr[32:33, :], sinv[32:33, :])
            st["sinr"] = sinr

        def emit_bc(st):
            sinr = st["sinr"]
            bc = sb_psum.tile([128, 512], f32, tag="sb", name="bc")
            # Proper open/close pairing: MM1 opens (rows 0:64 = sinv_A,
            # 64:128 = 0), MM2 accumulates sinv_B and closes.  No
            # skip_group_check: the sim/dep group state must see the close,
            # else readers observe an open group.
            nc.tensor.matmul(bc[:], maskAr[0:1, :], sinr[0:1, :],
                             start=True, stop=False)
            nc.tensor.matmul(bc[:], maskBr[32:33, :], sinr[32:33, :],
                             start=False, stop=True)
            st["bc"] = bc

        def emit_ot(st):
            p, ch = st["p"], st["ch"]
            nsl = slice(512 * ch, 512 * (ch + 1))
            nc.vector.tensor_copy(OT[p][:, nsl], st["o_ps"][:])
            nc.vector.tensor_tensor(OT[p][:, nsl], OT[p][:, nsl],
                                    st["bc"][:], op=OP.mult)

        def emit_pj_nt(nt, tail=False):
            # output projection + bias + store for one 128-query n-tile.
            # pj rides the sb pool's 2-slot rotation (with s_t/bc and the
            # woven chains): slots alternate so chain+drain pipeline 2-deep
            # and the S pool stays a pure QK double-buffer.
            pj = sb_psum.tile([128, 512], f32, tag="sb", name="pj")
            for p2 in range(KT):
                nc.tensor.matmul(
                    pj[:], OT[p2][:, 128 * nt:128 * (nt + 1)],
                    pw[p2][:],
                    start=(p2 == 0), stop=(p2 == KT - 1))
            ob = opool.tile([128, 512], bf16, tag="ob", name="ob")
            nc.vector.tensor_tensor(ob[:], pj[:], pb_bc[:],
                                    op=OP.add)
            nc.sync.dma_start(out_d.ap()[128 * nt:128 * (nt + 1), :],
                              ob[:])

        actions = {}     # mt -> list of thunks, for the current chunk
        nweave = 0       # round-robin counter for woven-chain psum slots
        n_ch = 0 if KSTAGE < 2 else min(KSTAGE - 1, NCH)
        for ch in range(n_ch):
            for p in range(KT):
                st = {"p": p, "ch": ch,
                      "o_ps": O_psum.tile([128, 512], f32, tag="o",
                                          name="o_ps"),
                      "accE": ss_pool.tile([128, 1024], bf16, tag="accE",
                                           name="accE"),
                      "accO": ss_pool.tile([128, 1024], bf16, tag="accO",
                                           name="accO")}
                nsl = slice(512 * ch, 512 * (ch + 1))
                for mt in range(MT):
                    for fn in actions.pop(mt, ()):
                        fn()
                    msl = slice(128 * mt, 128 * (mt + 1))
                    S_ps = S_psum.tile([128, 1024], f32, tag="S",
                                       name="S_ps")
                    nc.tensor.matmul(
                        S_ps[:, 0:512],
                        kTt[p][0:64, msl], qT[p][0:64, nsl],
                        start=True, stop=True, tile_position=(0, 0))
                    nc.tensor.matmul(
                        S_ps[:, 512:1024],
                        kTt[p][64:128, msl], qT[p][64:128, nsl],
                        start=True, stop=True, tile_position=(64, 0))
                    pt = ppool.tile([128, 1024], bf16, tag="pt", name="pt")
                    nc.scalar.activation(pt[:], S_ps[:], AF.Exp,
                                         bias=zero_t[:])
                    if pending_pv is not None:
                        emit_pv(pending_pv)
                    pending_pv = (p, mt, pt, st["o_ps"])
                    if mt == 0:
                        nc.vector.tensor_copy(st["accE"][:], pt[:])
                    elif mt == 1:
                        nc.vector.tensor_copy(st["accO"][:], pt[:])
                    elif mt % 2 == 0:
                        nc.gpsimd.tensor_tensor(st["accE"][:], st["accE"][:],
                                                pt[:], op=OP.add)
                    else:
                        nc.vector.tensor_tensor(st["accO"][:], st["accO"][:],
                                                pt[:], op=OP.add)
                    if WEAVE and ch == 0 and p == 0 and mt < 6:
                        emit_vchain(mt + 2,
                                    ps_fn=lambda vh, b=nweave:
                                    chain_tile(b + vh))
                        nweave += 2
                    if WEAVE and ch == 0 and p < KT - 1 and mt == 5:
                        emit_kchain(p + 1,
                                    ps_fn=lambda c2, b=nweave:
                                    chain_tile(b + c2))
                        nweave += 2
                # Epilogue deferred into the next group's mt stream, one
                # stage per slot so no PE stall and no psum-slot pile-up:
                # smm+recip@1, bc@2, OT@3, pj@4..7 (pj only after the
                # chunk's last pair).
                actions = {
                    1: [lambda st=st: emit_smm(st),
                        lambda st=st: emit_recip(st)],
                    2: [lambda st=st: emit_bc(st)],
                    3: [lambda st=st: emit_ot(st)],
                }
                if p == KT - 1:
                    tail = (ch == n_ch - 1)
                    for i, nt in enumerate(range(4 * ch, 4 * ch + 4)):
                        actions[4 + i] = [
                            lambda nt=nt, tail=tail: emit_pj_nt(nt, tail)]
        # drain: last chunk's PV + epilogue + all projection groups
        if pending_pv is not None:
            emit_pv(pending_pv)
        for mt in sorted(actions):
            for fn in actions[mt]:
                fn()
        close(opool, ppool, sinv_pool,
              sb_psum, O_psum, S_psum,
              ss_pool, OT_pool,
              kvp, zp, qt_pool, w_pool, consts)

    nc.compile()
    return nc


def _get_nc():
    if "nc" not in _cache:
        _cache["nc"] = _build_nc()
    return _cache["nc"]


def _make_in_maps(x, q_w, kv_w, proj_w, proj_b, sr_w, sr_b, ln_g, ln_b):
    import ml_dtypes
    bf = ml_dtypes.bfloat16

    x = np.asarray(x, np.float32)
    q_w = np.asarray(q_w, np.float32)
    kv_w = np.asarray(kv_w, np.float32)
    proj_w = np.asarray(proj_w, np.float32)
    g = np.asarray(ln_g, np.float32)
    lb = np.asarray(ln_b, np.float32)
    scale = HD ** -0.5

    q_wT = np.ascontiguousarray((q_w.T * scale).astype(bf))
    # fold ln_g into the kv weight columns (k = k_w @ (z*g + ln_b))
    k_wT = np.ascontiguousarray((kv_w[:C] * g[None, :]).T.astype(bf))
    v_wg = kv_w[C:] * g[None, :]
    v_wT_raw = v_wg.T  # [c_in, c_out=h*64+d]
    # packed per head pair p: [v_A|0] then [0|v_B] (for PV psum packing)
    v_wT = np.zeros((C, 2 * C), np.float32)
    for p in range(KT):
        v_wT[:, 256 * p:256 * p + 64] = v_wT_raw[:, 128 * p:128 * p + 64]
        v_wT[:, 256 * p + 192:256 * p + 256] = \
            v_wT_raw[:, 128 * p + 64:128 * p + 128]
    v_wT = np.ascontiguousarray(v_wT.astype(bf))
    p_wT = np.ascontiguousarray(proj_w.T.astype(bf))
    chan = np.zeros((C, 8), np.float32)
    sr_w = np.asarray(sr_w, np.float32)
    for di in range(2):
        for dj in range(2):
            chan[:, di * 2 + dj] = sr_w[:, 0, di, dj]
    chan[:, 4] = np.asarray(sr_b, np.float32)
    # ln_b folds: K-side bias is killed by softmax shift-invariance; the
    # V-side bias contributes exactly v_w@ln_b per channel (sum(P)/s == 1),
    # which commutes through the projection into an output bias.
    vb = kv_w[C:] @ lb
    p_b = (np.asarray(proj_b, np.float32) + proj_w @ vb).reshape(1, C)

    in_maps = []
    for d in range(8):
        b, half = d // 2, d % 2
        xbT = x[b].T  # [C, N]
        xCHW = xbT.reshape(C, 64, 64)
        planes = np.stack([xCHW[:, di::2, dj::2].reshape(C, M)
                           for di in range(2) for dj in range(2)], axis=1)
        xc = np.ascontiguousarray(planes.reshape(C, 4 * M).astype(bf))
        in_maps.append({
            "xc": xc,
            "xqT": np.ascontiguousarray(
                xbT[:, half * NHALF:(half + 1) * NHALF].astype(bf)),
            "q_wT": q_wT, "k_wT": k_wT, "v_wT": v_wT, "p_wT": p_wT,
            "chan": chan, "p_b": p_b,
        })
    return in_maps


def _assemble(res):
    out = np.empty((B, N, C), np.float32)
    for d in range(8):
        b, half = d // 2, d % 2
        out[b, half * NHALF:(half + 1) * NHALF, :] = res.results[d]["out"]
    return out


def kernel(x, q_w, kv_w, proj_w, proj_b, sr_w, sr_b, ln_g, ln_b, H, W):
    from concourse.bass_utils import run_bass_kernel_spmd

    nc = _get_nc()
    in_maps = _make_in_maps(x, q_w, kv_w, proj_w, proj_b, sr_w, sr_b,
                            ln_g, ln_b)
    res = run_bass_kernel_spmd(nc, in_maps, core_ids=list(range(8)))
    return _assemble(res)

